# revision 19
# baseline (speedup 1.0000x reference)
"""Bidirectional Mamba block (nn_Block_bi_mamba) Trainium2 Bass kernel.

Sharding: 8 cores = (batch b in {0,1}) x (d_inner quarter dq in {0..3}).
Each core computes, for its batch and both scan directions, the full
in_proj+conv (folded into PE matmuls) and x_proj (contracts over all 512
channels), the selective scan for its own 128 channels, and the out_proj
partial product [256, L]. The host sums the 4 partials per batch and
adds the residual x. The d_inner axis is permuted per core so the core's
own channel block is always channel-tile 0, making the device program
identical across cores (SPMD) with only input data differing.

Scan cluster runs in a state-major layout: SBUF partition p = 16 states
x 8 channels (channel group g covers channels 8g..8g+7). Per group the
DVE does exactly three ops (dBu mult, tensor_tensor_scan, C mult); the
16-state reduction y = sum_n C_n*h_n + D*u happens on the PE as PSUM-
accumulated selector/diagonal matmuls, keeping the add chain off the
bottleneck DVE. dt/du are computed channel-major once, round-tripped
through DRAM, and replicated across partitions by broadcast DMA. The
causal depthwise conv + in_proj is folded into PE matmuls with silu
fused into the PSUM-drain activation. All scan-cluster data is fp16
(near-1.0 resolution for dA that bf16 lacks; 2-byte operands run DVE
tensor ops at 2x).

Self-contained: hardcodes all shapes; no sibling imports.
"""
import numpy as np
import ml_dtypes
from contextlib import ExitStack

import concourse.bacc as bacc
import concourse.bass as bass
import concourse.tile as tile
from concourse import mybir
from concourse.bass_utils import run_bass_kernel_spmd
from concourse.alu_op_type import AluOpType as CCE

bf = ml_dtypes.bfloat16
FP32 = mybir.dt.float32
BF16 = mybir.dt.bfloat16
FP16 = mybir.dt.float16

B, L = 2, 4096
LC = 2048
NCH = L // LC
NSUB = LC // 512
N = 16
AOP = mybir.AluOpType
AF = mybir.ActivationFunctionType

# wconst fp32 [128, 44] columns
COL = {"A_f": 0, "A_b": 16, "cb_f": 32, "cb_b": 36, "Dd_f": 40,
       "Dd_b": 41, "dtb_f": 42, "dtb_b": 43}


def _bcast_ap(tensor_ap, row0, row_stride_rows, pdims, lc):
    """DRAM source AP replicating rows across 128 partitions.

    pdims: list of [stride_rows, size] partition dims (product 128),
    strides given in rows of length `lc`."""
    base = tensor_ap[row0:row0 + 1, 0:lc]
    ap = [[s * lc, n] for (s, n) in pdims] + [[1, lc]]
    return bass.AP(tensor=base.tensor, offset=base.offset, ap=ap)


def build_program(tc, ins, outs):
    nc = tc.nc
    with ExitStack() as ctx:
        wp = ctx.enter_context(tc.tile_pool(name="wp", bufs=1))
        big = ctx.enter_context(tc.tile_pool(name="big", bufs=1))
        work = ctx.enter_context(tc.tile_pool(name="work", bufs=1))
        scanp = ctx.enter_context(tc.tile_pool(name="scanp", bufs=2))
        ps = ctx.enter_context(tc.tile_pool(name="ps", bufs=2, space="PSUM"))
        dramp = ctx.enter_context(tc.tile_pool(name="dramp", bufs=2,
                                               space="DRAM"))

        # ---- weights ----
        wconst = wp.tile([128, 44], FP32, tag="wconst")
        nc.sync.dma_start(out=wconst, in_=ins["wconst"])

        wconv = wp.tile([128, 16 * 512], BF16, tag="wconv")
        nc.sync.dma_start(out=wconv, in_=ins["wconvP"])
        wz = wp.tile([128, 256], BF16, tag="wz")
        nc.sync.dma_start(out=wz, in_=ins["wzP"])
        outw = wp.tile([128, 256], FP16, tag="outw")
        nc.sync.dma_start(out=outw, in_=ins["outWT"])
        xpro = wp.tile([128, 384], FP16, tag="xpro")
        nc.sync.dma_start(out=xpro, in_=ins["xprojP"])
        dtprojp = wp.tile([16, 256], FP16, tag="dtprojp")
        nc.sync.dma_start(out=dtprojp, in_=ins["dtprojp"])
        selR = wp.tile([128, 16 * 128], FP16, tag="selR")
        nc.sync.dma_start(out=selR, in_=ins["selR"])
        selD = wp.tile([128, 2 * 128], FP16, tag="selD")
        nc.sync.dma_start(out=selD, in_=ins["selD"])
        carry = wp.tile([128, 32], FP32, tag="carry")

        # ---- persistent buffers ----
        xtp = [big.tile([128, L + 6], BF16, tag=f"xtp{kt}", name=f"xtp{kt}")
               for kt in range(2)]
        for kt in range(2):
            nc.sync.dma_start(out=xtp[kt],
                              in_=ins["xT16p"][kt * 128:(kt + 1) * 128])
        zs_all = big.tile([128, L], FP16, tag="zs")
        y_ball = big.tile([128, L], FP16, tag="yball")  # b-dir y, k-space

        # ---- phase Z (emitted as a function; interleaved below) ----
        def phase_z(c):
            for nsub in range(NSUB):
                pt = ps.tile([128, 512], FP32, tag="ps_conv")
                for kt in range(2):
                    nc.tensor.matmul(
                        pt, wz[:, kt * 128:(kt + 1) * 128],
                        xtp[kt][:, 3 + c * LC + nsub * 512:
                                3 + c * LC + (nsub + 1) * 512],
                        start=(kt == 0), stop=(kt == 1))
                nc.scalar.activation(
                    out=zs_all[:, c * LC + nsub * 512:
                               c * LC + (nsub + 1) * 512],
                    in_=pt, func=AF.Silu, bias=0.0, scale=1.0)

        combos = ([("b", c) for c in range(NCH - 1, -1, -1)]
                  + [("f", c) for c in range(NCH)])

        def front_end(d, c):
            """conv+silu -> xc; x_proj -> dbl + DRAM; dt/du -> DRAM."""
            dcol = 0 if d == "f" else 1
            base = 0 if d == "f" else 3
            cb0 = COL[f"cb_{d}"]
            xc = [work.tile([128, LC], FP16, tag=f"xc{t}", name=f"xc{t}",
                            bufs=(2 if t == 0 else 1)) for t in range(4)]
            dbl = work.tile([48, LC], FP16, tag="dbl", bufs=2)
            dt = work.tile([128, LC], FP16, tag="dt", bufs=1)
            # nsub-major so dt is ready as early as possible (the next
            # combo's du-mult sits in the DVE stream waiting for it)
            for nsub in range(NSUB):
                sl = slice(nsub * 512, (nsub + 1) * 512)
                for mt in range(4):
                    pt = ps.tile([128, 512], FP32, tag="ps_conv")
                    ns0 = c * LC + nsub * 512 + base
                    for idx, (k, kt) in enumerate(
                            (k, kt) for k in range(4) for kt in range(2)):
                        seg = ((dcol * 4 + k) * 2 + kt) * 512
                        nc.tensor.matmul(
                            pt, wconv[:, seg + mt * 128:seg + (mt + 1) * 128],
                            xtp[kt][:, ns0 + k:ns0 + k + 512],
                            start=(idx == 0), stop=(idx == 7))
                    # xc = silu(psum + conv_b) in one act
                    nc.scalar.activation(
                        out=xc[mt][:, sl], in_=pt, func=AF.Silu,
                        bias=wconst[:, cb0 + mt:cb0 + mt + 1], scale=1.0)
                pj = ps.tile([48, 512], FP32, tag="ps_xp", bufs=1)
                for kt in range(4):
                    nc.tensor.matmul(
                        pj, xpro[:, kt * 96 + 48 * dcol:
                                 kt * 96 + 48 * (dcol + 1)],
                        xc[kt][:, sl],
                        start=(kt == 0), stop=(kt == 3))
                nc.scalar.copy(out=dbl[:, sl], in_=pj)
                pt = ps.tile([128, 512], FP32, tag="ps_dt", bufs=1)
                nc.tensor.matmul(
                    pt, dtprojp[:, dcol * 128:(dcol + 1) * 128],
                    dbl[0:16, sl],
                    start=True, stop=True)
                esub = work.tile([128, 512], FP16, tag="esub", bufs=1)
                nc.scalar.activation(
                    out=esub, in_=pt, func=AF.Exp,
                    bias=wconst[:, COL[f"dtb_{d}"]:COL[f"dtb_{d}"] + 1],
                    scale=1.0)
                nc.scalar.activation(
                    out=dt[:, sl], in_=esub,
                    func=AF.Ln, bias=1.0, scale=1.0)
            scratch = dramp.tile([32, LC], FP16, tag="bcdram")
            nc.sync.dma_start(out=scratch, in_=dbl[16:48, :])

            dtdram = dramp.tile([128, LC], FP16, tag="dtdram")
            nc.sync.dma_start(out=dtdram, in_=dt)
            st = {"xc0": xc[0], "dt": dt, "scratch": scratch,
                  "dtdram": dtdram}

            def mid_emit():
                """du mult + DRAM write, deferred so the DVE stream of the
                previous combo's scan cluster is not blocked on dt."""
                du = work.tile([128, LC], FP16, tag="du", bufs=1)
                nc.vector.tensor_tensor(du, st["dt"], st["xc0"], AOP.mult)
                dudram = dramp.tile([128, LC], FP16, tag="dudram")
                nc.sync.dma_start(out=dudram, in_=du)
                st["dudram"] = dudram

            st["mid_emit"] = mid_emit
            return st

        def scan_phase(d, c, st, first, next_mid=None, post_emit=None):
            dcol = 0 if d == "f" else 1
            rev = (lambda ap: ap[:, ::-1]) if d == "b" else (lambda ap: ap)
            cr = slice(c * LC, (c + 1) * LC)
            scratch = st["scratch"]

            # B/C rows replicated 8x: partition p=(n,j) <- row n
            Bt = scanp.tile([128, LC], FP16, tag="Bt", bufs=2)
            nc.sync.dma_start(out=Bt, in_=_bcast_ap(
                scratch, 0, 1, [[1, 16], [0, 8]], LC))
            Ct = scanp.tile([128, LC], FP16, tag="Ct", bufs=2)
            nc.sync.dma_start(out=Ct, in_=_bcast_ap(
                scratch, 16, 1, [[1, 16], [0, 8]], LC))

            ypsum = ps.tile([128, LC], FP32, tag="ps_y", bufs=1)
            # D*u skip opens the accumulation group (start); b-dir psum is
            # in k-space (time-reversed), so feed u reversed.
            if d == "b":
                u_d = work.tile([128, LC], FP16, tag="xcr", bufs=1)
                nc.scalar.copy(out=u_d, in_=st["xc0"][:, ::-1])
            else:
                u_d = st["xc0"]
            for ns in range(NSUB):
                nc.tensor.matmul(
                    ypsum[:, ns * 512:(ns + 1) * 512],
                    selD[:, dcol * 128:(dcol + 1) * 128],
                    u_d[:, ns * 512:(ns + 1) * 512],
                    start=True, stop=False, skip_group_check=True)
            for g in range(16):
                if g == 12 and next_mid is not None:
                    next_mid()
                if g == 4 and post_emit is not None:
                    post_emit()
                # dt/du rows 8g..8g+7 replicated 16x: p=(n,j) <- row 8g+j
                dtr = scanp.tile([128, LC], FP16, tag="dtr", bufs=3)
                nc.sync.dma_start(out=dtr, in_=_bcast_ap(
                    st["dtdram"], 8 * g, 1, [[0, 16], [1, 8]], LC))
                dur = scanp.tile([128, LC], FP16, tag="dur", bufs=3)
                nc.sync.dma_start(out=dur, in_=_bcast_ap(
                    st["dudram"], 8 * g, 1, [[0, 16], [1, 8]], LC))

                dA = scanp.tile([128, LC], FP16, tag="dA", bufs=3)
                nc.scalar.activation(
                    out=dA, in_=dtr, func=AF.Exp, bias=0.0,
                    scale=wconst[:, COL[f"A_{d}"] + g:COL[f"A_{d}"] + g + 1])
                dBu = scanp.tile([128, LC], FP16, tag="dBu", bufs=3)
                nc.vector.tensor_tensor(dBu, dur, Bt, AOP.mult)
                h = scanp.tile([128, LC], FP16, tag="h", bufs=2)
                init = (0.0 if first
                        else carry[:, dcol * 16 + g:dcol * 16 + g + 1])
                nc.vector.tensor_tensor_scan(h, rev(dA), rev(dBu), init,
                                             AOP.mult, AOP.add)
                if first and NCH > 1:
                    nc.vector.tensor_copy(
                        out=carry[:, dcol * 16 + g:dcol * 16 + g + 1],
                        in_=h[:, LC - 1:LC])
                hC = scanp.tile([128, LC], FP16, tag="hC", bufs=3)
                nc.vector.tensor_tensor(hC, h, rev(Ct), AOP.mult)
                # PE: accumulate this group's 16 states into y channels
                for ns in range(NSUB):
                    nc.tensor.matmul(
                        ypsum[:, ns * 512:(ns + 1) * 512],
                        selR[:, g * 128:(g + 1) * 128],
                        hC[:, ns * 512:(ns + 1) * 512],
                        start=False, stop=(g == 15), skip_group_check=True)

            if d == "b":
                # keep k-space; f-tail reads it reversed
                for ns in range(NSUB):
                    nc.scalar.copy(
                        out=y_ball[:, c * LC + ns * 512:
                                   c * LC + (ns + 1) * 512],
                        in_=ypsum[:, ns * 512:(ns + 1) * 512])
            else:
                ysb = work.tile([128, LC], FP16, tag="ysb", bufs=2)
                for ns in range(NSUB):
                    nc.scalar.copy(out=ysb[:, ns * 512:(ns + 1) * 512],
                                   in_=ypsum[:, ns * 512:(ns + 1) * 512])
                ysum = work.tile([128, LC], FP16, tag="ytmp", bufs=2)
                nc.vector.tensor_tensor(ysum, ysb, y_ball[:, cr][:, ::-1],
                                        AOP.add)
                ygated = work.tile([128, LC], FP16, tag="ytmp", bufs=2)
                nc.vector.tensor_tensor(ygated, ysum, zs_all[:, cr], AOP.mult)
                # out_proj: psum regions cycle inside ypsum
                for mt in range(2):
                    osb = work.tile([128, LC], FP32, tag="osb")
                    for nsub in range(NSUB):
                        q = (mt * NSUB + nsub) % NSUB
                        po = ypsum[:, q * 512:(q + 1) * 512]
                        nc.tensor.matmul(
                            po, outw[:, mt * 128:(mt + 1) * 128],
                            ygated[:, nsub * 512:(nsub + 1) * 512],
                            start=True, stop=True)
                        nc.scalar.copy(
                            out=osb[:, nsub * 512:(nsub + 1) * 512], in_=po)
                    nc.sync.dma_start(
                        out=outs["attnT"][mt * 128:(mt + 1) * 128, cr],
                        in_=osb)

        # software pipeline: front_end one combo ahead of the scan phase;
        # du-mult of combo j+1 and phase-Z are emitted inside scan j's
        # group loop so they never head-block the DVE stream
        states = {}
        states[0] = front_end(*combos[0])
        states[0]["mid_emit"]()
        for j, (d, c) in enumerate(combos):
            if j + 1 < len(combos):
                states[j + 1] = front_end(*combos[j + 1])
                next_mid = states[j + 1]["mid_emit"]
            else:
                next_mid = None
            post = (lambda cc=j: phase_z(cc)) if j < NCH else None
            first = (j % NCH == 0)
            scan_phase(d, c, states.pop(j), first, next_mid=next_mid,
                       post_emit=post)


def build_nc():
    nc = bacc.Bacc("TRN2", target_bir_lowering=False, debug=False,
                   enable_asserts=False)
    ins = {}

    def inp(name, shape, dt):
        ins[name] = nc.dram_tensor(name, shape, dt,
                                   kind="ExternalInput").ap()

    inp("xT16p", [256, L + 6], BF16)
    inp("wconvP", [128, 16 * 512], BF16)
    inp("wzP", [128, 256], BF16)
    inp("outWT", [128, 256], FP16)
    inp("xprojP", [128, 384], FP16)
    inp("dtprojp", [16, 256], FP16)
    inp("wconst", [128, 44], FP32)
    inp("selR", [128, 16 * 128], FP16)
    inp("selD", [128, 2 * 128], FP16)
    outs = {"attnT": nc.dram_tensor("attnT", [256, L], FP32,
                                    kind="ExternalOutput").ap()}
    with tile.TileContext(nc) as tc:
        build_program(tc, ins, outs)
    nc.compile()
    return nc


def prep_core_inputs(inputs, b, dq):
    """Per-core input arrays; d_inner axis permuted so own block is first."""
    own = np.arange(dq * 128, (dq + 1) * 128)
    rest = np.array([i for i in range(512)
                     if not (dq * 128 <= i < (dq + 1) * 128)])
    perm = np.concatenate([own, rest])

    out = {}
    xT = inputs["x"][b].T.astype(np.float32)  # [256, L]
    xTp = np.zeros((256, L + 6), np.float32)
    xTp[:, 3:L + 3] = xT
    out["xT16p"] = xTp.astype(bf)

    w_inx = inputs["in_proj_w"][:512][perm].astype(np.float32)  # [512, 256]
    wconvP = np.zeros((128, 16 * 512), np.float32)
    for dcol, d in enumerate("fb"):
        cw = inputs[f"conv_w_{d}"][:, 0, :][perm].astype(np.float32)
        for k in range(4):
            tap = cw[:, k] if d == "f" else cw[:, 3 - k]
            WdkT = (tap[:, None] * w_inx).T     # [256, 512]
            for kt in range(2):
                seg = ((dcol * 4 + k) * 2 + kt) * 512
                wconvP[:, seg:seg + 512] = WdkT[kt * 128:(kt + 1) * 128]
    out["wconvP"] = wconvP.astype(bf)

    wz = inputs["in_proj_w"][512:1024][own].astype(np.float32)  # [128, 256]
    wzP = np.zeros((128, 256), np.float32)
    for kt in range(2):
        wzP[:, kt * 128:(kt + 1) * 128] = wz.T[kt * 128:(kt + 1) * 128]
    out["wzP"] = wzP.astype(bf)

    out["outWT"] = np.ascontiguousarray(
        inputs["out_proj_w"][:, own].T).astype(np.float16)  # [128, 256]

    xprojP = np.zeros((128, 384), np.float32)
    xpf = inputs["xproj_w_f"][:, perm].T  # [512, 48]
    xpb = inputs["xproj_w_b"][:, perm].T
    for kt in range(4):
        xprojP[:, kt * 96:kt * 96 + 48] = xpf[kt * 128:(kt + 1) * 128]
        xprojP[:, kt * 96 + 48:kt * 96 + 96] = xpb[kt * 128:(kt + 1) * 128]
    out["xprojP"] = xprojP.astype(np.float16)

    out["dtprojp"] = np.ascontiguousarray(np.concatenate(
        [inputs["dtproj_w_f"][own].T, inputs["dtproj_w_b"][own].T],
        axis=1)).astype(np.float16)  # [16, 256]

    # state-major helpers: partition p = n*8 + j  (n = state, j = chan%8)
    pn = np.arange(128) // 8
    pj = np.arange(128) % 8

    wconst = np.zeros((128, 44), np.float32)
    for i, d in enumerate("fb"):
        A = -np.exp(inputs[f"A_log_{d}"][own].astype(np.float64))  # [128, 16]
        for g in range(16):
            wconst[:, 16 * i + g] = A[8 * g + pj, pn]
        cb = inputs[f"conv_b_{d}"][perm]
        wconst[:, 32 + 4 * i:36 + 4 * i] = cb.reshape(4, 128).T
        wconst[:, 40 + i] = inputs[f"D_{d}"][own]
        wconst[:, 42 + i] = inputs[f"dtproj_b_{d}"][own]
    out["wconst"] = wconst

    selR = np.zeros((128, 16 * 128), np.float16)
    for g in range(16):
        selR[np.arange(128), g * 128 + 8 * g + pj] = 1.0
    out["selR"] = selR
    selD = np.zeros((128, 2 * 128), np.float16)
    for i, d in enumerate("fb"):
        selD[np.arange(128), i * 128 + np.arange(128)] = \
            inputs[f"D_{d}"][own].astype(np.float16)
    out["selD"] = selD
    return out


_CACHE = {}


def kernel(**inputs):
    inputs = {k: np.asarray(v) for k, v in inputs.items()}
    if "nc" not in _CACHE:
        _CACHE["nc"] = build_nc()
    nc = _CACHE["nc"]

    core_ids = list(range(8))
    in_maps = [prep_core_inputs(inputs, core // 4, core % 4)
               for core in core_ids]
    import os
    trace = os.environ.get("BASS_KERNEL_TRACE", "0") == "1"
    res = run_bass_kernel_spmd(nc, in_maps, core_ids, trace=trace)
    _CACHE["last_results"] = res

    x = inputs["x"].astype(np.float32)
    out = np.empty((B, L, 256), np.float32)
    for b in range(B):
        acc = np.zeros((256, L), np.float32)
        for dq in range(4):
            acc += res.results[4 * b + dq]["attnT"]
        out[b] = x[b] + acc.T
    return out.astype(np.float32)


# revision 21
# speedup vs baseline: 1.0097x; 1.0097x over previous
"""Bidirectional Mamba block (nn_Block_bi_mamba) Trainium2 Bass kernel.

Sharding: 8 cores = (batch b in {0,1}) x (d_inner quarter dq in {0..3}).
Each core computes, for its batch and both scan directions, the full
in_proj+conv (folded into PE matmuls) and x_proj (contracts over all 512
channels), the selective scan for its own 128 channels, and the out_proj
partial product [256, L]. The host sums the 4 partials per batch and
adds the residual x. The d_inner axis is permuted per core so the core's
own channel block is always channel-tile 0, making the device program
identical across cores (SPMD) with only input data differing.

Scan cluster runs in a state-major layout: SBUF partition p = 16 states
x 8 channels (channel group g covers channels 8g..8g+7). Per group the
DVE does exactly three ops (dBu mult, tensor_tensor_scan, C mult); the
16-state reduction y = sum_n C_n*h_n + D*u happens on the PE as PSUM-
accumulated selector/diagonal matmuls, keeping the add chain off the
bottleneck DVE. dt/du are computed channel-major once, round-tripped
through DRAM, and replicated across partitions by broadcast DMA. The
causal depthwise conv + in_proj is folded into PE matmuls with silu
fused into the PSUM-drain activation. All scan-cluster data is fp16
(near-1.0 resolution for dA that bf16 lacks; 2-byte operands run DVE
tensor ops at 2x).

Self-contained: hardcodes all shapes; no sibling imports.
"""
import numpy as np
import ml_dtypes
from contextlib import ExitStack

import concourse.bacc as bacc
import concourse.bass as bass
import concourse.tile as tile
from concourse import mybir
from concourse.bass_utils import run_bass_kernel_spmd
from concourse.alu_op_type import AluOpType as CCE

bf = ml_dtypes.bfloat16
FP32 = mybir.dt.float32
BF16 = mybir.dt.bfloat16
FP16 = mybir.dt.float16

B, L = 2, 4096
LC = 2048
NCH = L // LC
NSUB = LC // 512
N = 16
AOP = mybir.AluOpType
AF = mybir.ActivationFunctionType

# wconst fp32 [128, 44] columns
COL = {"A_f": 0, "A_b": 16, "cb_f": 32, "cb_b": 36, "Dd_f": 40,
       "Dd_b": 41, "dtb_f": 42, "dtb_b": 43}


def _bcast_ap(tensor_ap, row0, row_stride_rows, pdims, lc):
    """DRAM source AP replicating rows across 128 partitions.

    pdims: list of [stride_rows, size] partition dims (product 128),
    strides given in rows of length `lc`."""
    base = tensor_ap[row0:row0 + 1, 0:lc]
    ap = [[s * lc, n] for (s, n) in pdims] + [[1, lc]]
    return bass.AP(tensor=base.tensor, offset=base.offset, ap=ap)


def build_program(tc, ins, outs):
    nc = tc.nc
    with ExitStack() as ctx:
        wp = ctx.enter_context(tc.tile_pool(name="wp", bufs=1))
        big = ctx.enter_context(tc.tile_pool(name="big", bufs=1))
        work = ctx.enter_context(tc.tile_pool(name="work", bufs=1))
        scanp = ctx.enter_context(tc.tile_pool(name="scanp", bufs=2))
        ps = ctx.enter_context(tc.tile_pool(name="ps", bufs=2, space="PSUM"))
        dramp = ctx.enter_context(tc.tile_pool(name="dramp", bufs=2,
                                               space="DRAM"))

        # ---- weights ----
        wconst = wp.tile([128, 44], FP32, tag="wconst")
        nc.sync.dma_start(out=wconst, in_=ins["wconst"])

        wconv = wp.tile([128, 16 * 512], BF16, tag="wconv")
        nc.sync.dma_start(out=wconv, in_=ins["wconvP"])
        wz = wp.tile([128, 256], BF16, tag="wz")
        nc.sync.dma_start(out=wz, in_=ins["wzP"])
        outw = wp.tile([128, 256], FP16, tag="outw")
        nc.sync.dma_start(out=outw, in_=ins["outWT"])
        xpro = wp.tile([128, 384], FP16, tag="xpro")
        nc.sync.dma_start(out=xpro, in_=ins["xprojP"])
        dtprojp = wp.tile([16, 256], FP16, tag="dtprojp")
        nc.sync.dma_start(out=dtprojp, in_=ins["dtprojp"])
        selR = wp.tile([128, 16 * 128], FP16, tag="selR")
        nc.sync.dma_start(out=selR, in_=ins["selR"])
        selD = wp.tile([128, 2 * 128], FP16, tag="selD")
        nc.sync.dma_start(out=selD, in_=ins["selD"])
        carry = wp.tile([128, 32], FP32, tag="carry")

        # ---- persistent buffers ----
        xtp = [big.tile([128, L + 6], BF16, tag=f"xtp{kt}", name=f"xtp{kt}")
               for kt in range(2)]
        for kt in range(2):
            nc.sync.dma_start(out=xtp[kt],
                              in_=ins["xT16p"][kt * 128:(kt + 1) * 128])
        zs_all = big.tile([128, L], FP16, tag="zs")
        y_ball = big.tile([128, L], FP16, tag="yball")  # b-dir y, k-space

        # ---- phase Z (emitted as a function; interleaved below) ----
        def phase_z(c):
            for nsub in range(NSUB):
                pt = ps.tile([128, 512], FP32, tag="ps_conv")
                for kt in range(2):
                    nc.tensor.matmul(
                        pt, wz[:, kt * 128:(kt + 1) * 128],
                        xtp[kt][:, 3 + c * LC + nsub * 512:
                                3 + c * LC + (nsub + 1) * 512],
                        start=(kt == 0), stop=(kt == 1))
                nc.scalar.activation(
                    out=zs_all[:, c * LC + nsub * 512:
                               c * LC + (nsub + 1) * 512],
                    in_=pt, func=AF.Silu, bias=0.0, scale=1.0)

        combos = ([("b", c) for c in range(NCH - 1, -1, -1)]
                  + [("f", c) for c in range(NCH)])

        def front_end(d, c):
            """conv+silu -> xc; x_proj -> dbl + DRAM; dt/du -> DRAM."""
            dcol = 0 if d == "f" else 1
            base = 0 if d == "f" else 3
            cb0 = COL[f"cb_{d}"]
            xc = [work.tile([128, LC], FP16, tag=f"xc{t}", name=f"xc{t}",
                            bufs=(2 if t == 0 else 1)) for t in range(4)]
            dbl = work.tile([48, LC], FP16, tag="dbl", bufs=1)
            dt = work.tile([128, LC], FP16, tag="dt", bufs=1)
            # nsub-major so dt is ready as early as possible (the next
            # combo's du-mult sits in the DVE stream waiting for it)
            for nsub in range(NSUB):
                sl = slice(nsub * 512, (nsub + 1) * 512)
                for mt in range(4):
                    pt = ps.tile([128, 512], FP32, tag="ps_conv")
                    ns0 = c * LC + nsub * 512 + base
                    for idx, (k, kt) in enumerate(
                            (k, kt) for k in range(4) for kt in range(2)):
                        seg = ((dcol * 4 + k) * 2 + kt) * 512
                        nc.tensor.matmul(
                            pt, wconv[:, seg + mt * 128:seg + (mt + 1) * 128],
                            xtp[kt][:, ns0 + k:ns0 + k + 512],
                            start=(idx == 0), stop=(idx == 7))
                    # xc = silu(psum + conv_b) in one act
                    nc.scalar.activation(
                        out=xc[mt][:, sl], in_=pt, func=AF.Silu,
                        bias=wconst[:, cb0 + mt:cb0 + mt + 1], scale=1.0)
                pj = ps.tile([48, 512], FP32, tag="ps_xp", bufs=1)
                for kt in range(4):
                    nc.tensor.matmul(
                        pj, xpro[:, kt * 96 + 48 * dcol:
                                 kt * 96 + 48 * (dcol + 1)],
                        xc[kt][:, sl],
                        start=(kt == 0), stop=(kt == 3))
                nc.scalar.copy(out=dbl[:, sl], in_=pj)
                pt = ps.tile([128, 512], FP32, tag="ps_dt", bufs=1)
                nc.tensor.matmul(
                    pt, dtprojp[:, dcol * 128:(dcol + 1) * 128],
                    dbl[0:16, sl],
                    start=True, stop=True)
                esub = work.tile([128, 512], FP16, tag="esub", bufs=1)
                nc.scalar.activation(
                    out=esub, in_=pt, func=AF.Exp,
                    bias=wconst[:, COL[f"dtb_{d}"]:COL[f"dtb_{d}"] + 1],
                    scale=1.0)
                nc.scalar.activation(
                    out=dt[:, sl], in_=esub,
                    func=AF.Ln, bias=1.0, scale=1.0)
            scratch = dramp.tile([32, LC], FP16, tag="bcdram")
            nc.sync.dma_start(out=scratch, in_=dbl[16:48, :])

            dtdram = dramp.tile([128, LC], FP16, tag="dtdram")
            nc.sync.dma_start(out=dtdram, in_=dt)
            st = {"xc0": xc[0], "dt": dt, "scratch": scratch,
                  "dtdram": dtdram}

            def mid_emit():
                """du mult + DRAM write, deferred so the DVE stream of the
                previous combo's scan cluster is not blocked on dt."""
                du = work.tile([128, LC], FP16, tag="du", bufs=1)
                nc.vector.tensor_tensor(du, st["dt"], st["xc0"], AOP.mult)
                dudram = dramp.tile([128, LC], FP16, tag="dudram")
                nc.sync.dma_start(out=dudram, in_=du)
                st["dudram"] = dudram

            st["mid_emit"] = mid_emit
            return st

        def scan_phase(d, c, st, first, next_mid=None, post_emit=None):
            dcol = 0 if d == "f" else 1
            rev = (lambda ap: ap[:, ::-1]) if d == "b" else (lambda ap: ap)
            cr = slice(c * LC, (c + 1) * LC)
            scratch = st["scratch"]

            # B/C rows replicated 8x: partition p=(n,j) <- row n
            Bt = scanp.tile([128, LC], FP16, tag="Bt", bufs=2)
            nc.sync.dma_start(out=Bt, in_=_bcast_ap(
                scratch, 0, 1, [[1, 16], [0, 8]], LC))
            Ct = scanp.tile([128, LC], FP16, tag="Ct", bufs=2)
            nc.sync.dma_start(out=Ct, in_=_bcast_ap(
                scratch, 16, 1, [[1, 16], [0, 8]], LC))

            ypsum = ps.tile([128, LC], FP32, tag="ps_y", bufs=1)
            # D*u skip opens the accumulation group (start); b-dir psum is
            # in k-space (time-reversed), so feed u reversed.
            if d == "b":
                u_d = work.tile([128, LC], FP16, tag="xcr", bufs=1)
                nc.scalar.copy(out=u_d, in_=st["xc0"][:, ::-1])
            else:
                u_d = st["xc0"]
            for ns in range(NSUB):
                nc.tensor.matmul(
                    ypsum[:, ns * 512:(ns + 1) * 512],
                    selD[:, dcol * 128:(dcol + 1) * 128],
                    u_d[:, ns * 512:(ns + 1) * 512],
                    start=True, stop=False, skip_group_check=True)
            for g in range(16):
                if g == 12 and next_mid is not None:
                    next_mid()
                if g == 4 and post_emit is not None:
                    post_emit()
                # dt/du rows 8g..8g+7 replicated 16x: p=(n,j) <- row 8g+j
                dtr = scanp.tile([128, LC], FP16, tag="dtr", bufs=5)
                nc.sync.dma_start(out=dtr, in_=_bcast_ap(
                    st["dtdram"], 8 * g, 1, [[0, 16], [1, 8]], LC))
                dur = scanp.tile([128, LC], FP16, tag="dur", bufs=2)
                nc.sync.dma_start(out=dur, in_=_bcast_ap(
                    st["dudram"], 8 * g, 1, [[0, 16], [1, 8]], LC))

                dA = scanp.tile([128, LC], FP16, tag="dA", bufs=5)
                with tc.high_priority():
                    nc.scalar.activation(
                        out=dA, in_=dtr, func=AF.Exp, bias=0.0,
                        scale=wconst[:, COL[f"A_{d}"] + g:
                                     COL[f"A_{d}"] + g + 1])
                dBu = scanp.tile([128, LC], FP16, tag="dBu", bufs=3)
                nc.vector.tensor_tensor(dBu, dur, Bt, AOP.mult)
                h = scanp.tile([128, LC], FP16, tag="h", bufs=2)
                init = (0.0 if first
                        else carry[:, dcol * 16 + g:dcol * 16 + g + 1])
                nc.vector.tensor_tensor_scan(h, rev(dA), rev(dBu), init,
                                             AOP.mult, AOP.add)
                if first and NCH > 1:
                    nc.vector.tensor_copy(
                        out=carry[:, dcol * 16 + g:dcol * 16 + g + 1],
                        in_=h[:, LC - 1:LC])
                hC = scanp.tile([128, LC], FP16, tag="hC", bufs=3)
                nc.vector.tensor_tensor(hC, h, rev(Ct), AOP.mult)
                # PE: accumulate this group's 16 states into y channels
                for ns in range(NSUB):
                    nc.tensor.matmul(
                        ypsum[:, ns * 512:(ns + 1) * 512],
                        selR[:, g * 128:(g + 1) * 128],
                        hC[:, ns * 512:(ns + 1) * 512],
                        start=False, stop=(g == 15), skip_group_check=True)

            if d == "b":
                # keep k-space; f-tail reads it reversed
                with tc.high_priority():
                    for ns in range(NSUB):
                        nc.scalar.copy(
                            out=y_ball[:, c * LC + ns * 512:
                                       c * LC + (ns + 1) * 512],
                            in_=ypsum[:, ns * 512:(ns + 1) * 512])
            else:
                ysb = work.tile([128, LC], FP16, tag="ysb", bufs=2)
                with tc.high_priority():
                    for ns in range(NSUB):
                        nc.scalar.copy(out=ysb[:, ns * 512:(ns + 1) * 512],
                                       in_=ypsum[:, ns * 512:(ns + 1) * 512])
                nc.vector.tensor_tensor(ysb, ysb, y_ball[:, cr][:, ::-1],
                                        AOP.add)
                nc.vector.tensor_tensor(ysb, ysb, zs_all[:, cr], AOP.mult)
                ygated = ysb
                # out_proj: psum regions cycle inside ypsum
                for mt in range(2):
                    osb = work.tile([128, LC], FP32, tag="osb")
                    for nsub in range(NSUB):
                        q = (mt * NSUB + nsub) % NSUB
                        po = ypsum[:, q * 512:(q + 1) * 512]
                        nc.tensor.matmul(
                            po, outw[:, mt * 128:(mt + 1) * 128],
                            ygated[:, nsub * 512:(nsub + 1) * 512],
                            start=True, stop=True)
                        nc.scalar.copy(
                            out=osb[:, nsub * 512:(nsub + 1) * 512], in_=po)
                    nc.sync.dma_start(
                        out=outs["attnT"][mt * 128:(mt + 1) * 128, cr],
                        in_=osb)

        # software pipeline: front_end one combo ahead of the scan phase;
        # du-mult of combo j+1 and phase-Z are emitted inside scan j's
        # group loop so they never head-block the DVE stream
        states = {}
        states[0] = front_end(*combos[0])
        states[0]["mid_emit"]()
        for j, (d, c) in enumerate(combos):
            if j + 1 < len(combos):
                states[j + 1] = front_end(*combos[j + 1])
                next_mid = states[j + 1]["mid_emit"]
            else:
                next_mid = None
            post = (lambda cc=j: phase_z(cc)) if j < NCH else None
            first = (j % NCH == 0)
            scan_phase(d, c, states.pop(j), first, next_mid=next_mid,
                       post_emit=post)


def build_nc():
    nc = bacc.Bacc("TRN2", target_bir_lowering=False, debug=False,
                   enable_asserts=False)
    ins = {}

    def inp(name, shape, dt):
        ins[name] = nc.dram_tensor(name, shape, dt,
                                   kind="ExternalInput").ap()

    inp("xT16p", [256, L + 6], BF16)
    inp("wconvP", [128, 16 * 512], BF16)
    inp("wzP", [128, 256], BF16)
    inp("outWT", [128, 256], FP16)
    inp("xprojP", [128, 384], FP16)
    inp("dtprojp", [16, 256], FP16)
    inp("wconst", [128, 44], FP32)
    inp("selR", [128, 16 * 128], FP16)
    inp("selD", [128, 2 * 128], FP16)
    outs = {"attnT": nc.dram_tensor("attnT", [256, L], FP32,
                                    kind="ExternalOutput").ap()}
    with tile.TileContext(nc) as tc:
        build_program(tc, ins, outs)
    nc.compile()
    return nc


def prep_core_inputs(inputs, b, dq):
    """Per-core input arrays; d_inner axis permuted so own block is first."""
    own = np.arange(dq * 128, (dq + 1) * 128)
    rest = np.array([i for i in range(512)
                     if not (dq * 128 <= i < (dq + 1) * 128)])
    perm = np.concatenate([own, rest])

    out = {}
    xT = inputs["x"][b].T.astype(np.float32)  # [256, L]
    xTp = np.zeros((256, L + 6), np.float32)
    xTp[:, 3:L + 3] = xT
    out["xT16p"] = xTp.astype(bf)

    w_inx = inputs["in_proj_w"][:512][perm].astype(np.float32)  # [512, 256]
    wconvP = np.zeros((128, 16 * 512), np.float32)
    for dcol, d in enumerate("fb"):
        cw = inputs[f"conv_w_{d}"][:, 0, :][perm].astype(np.float32)
        for k in range(4):
            tap = cw[:, k] if d == "f" else cw[:, 3 - k]
            WdkT = (tap[:, None] * w_inx).T     # [256, 512]
            for kt in range(2):
                seg = ((dcol * 4 + k) * 2 + kt) * 512
                wconvP[:, seg:seg + 512] = WdkT[kt * 128:(kt + 1) * 128]
    out["wconvP"] = wconvP.astype(bf)

    wz = inputs["in_proj_w"][512:1024][own].astype(np.float32)  # [128, 256]
    wzP = np.zeros((128, 256), np.float32)
    for kt in range(2):
        wzP[:, kt * 128:(kt + 1) * 128] = wz.T[kt * 128:(kt + 1) * 128]
    out["wzP"] = wzP.astype(bf)

    out["outWT"] = np.ascontiguousarray(
        inputs["out_proj_w"][:, own].T).astype(np.float16)  # [128, 256]

    xprojP = np.zeros((128, 384), np.float32)
    xpf = inputs["xproj_w_f"][:, perm].T  # [512, 48]
    xpb = inputs["xproj_w_b"][:, perm].T
    for kt in range(4):
        xprojP[:, kt * 96:kt * 96 + 48] = xpf[kt * 128:(kt + 1) * 128]
        xprojP[:, kt * 96 + 48:kt * 96 + 96] = xpb[kt * 128:(kt + 1) * 128]
    out["xprojP"] = xprojP.astype(np.float16)

    out["dtprojp"] = np.ascontiguousarray(np.concatenate(
        [inputs["dtproj_w_f"][own].T, inputs["dtproj_w_b"][own].T],
        axis=1)).astype(np.float16)  # [16, 256]

    # state-major helpers: partition p = n*8 + j  (n = state, j = chan%8)
    pn = np.arange(128) // 8
    pj = np.arange(128) % 8

    wconst = np.zeros((128, 44), np.float32)
    for i, d in enumerate("fb"):
        A = -np.exp(inputs[f"A_log_{d}"][own].astype(np.float64))  # [128, 16]
        for g in range(16):
            wconst[:, 16 * i + g] = A[8 * g + pj, pn]
        cb = inputs[f"conv_b_{d}"][perm]
        wconst[:, 32 + 4 * i:36 + 4 * i] = cb.reshape(4, 128).T
        wconst[:, 40 + i] = inputs[f"D_{d}"][own]
        wconst[:, 42 + i] = inputs[f"dtproj_b_{d}"][own]
    out["wconst"] = wconst

    selR = np.zeros((128, 16 * 128), np.float16)
    for g in range(16):
        selR[np.arange(128), g * 128 + 8 * g + pj] = 1.0
    out["selR"] = selR
    selD = np.zeros((128, 2 * 128), np.float16)
    for i, d in enumerate("fb"):
        selD[np.arange(128), i * 128 + np.arange(128)] = \
            inputs[f"D_{d}"][own].astype(np.float16)
    out["selD"] = selD
    return out


_CACHE = {}


def kernel(**inputs):
    inputs = {k: np.asarray(v) for k, v in inputs.items()}
    if "nc" not in _CACHE:
        _CACHE["nc"] = build_nc()
    nc = _CACHE["nc"]

    core_ids = list(range(8))
    in_maps = [prep_core_inputs(inputs, core // 4, core % 4)
               for core in core_ids]
    import os
    trace = os.environ.get("BASS_KERNEL_TRACE", "0") == "1"
    res = run_bass_kernel_spmd(nc, in_maps, core_ids, trace=trace)
    _CACHE["last_results"] = res

    x = inputs["x"].astype(np.float32)
    out = np.empty((B, L, 256), np.float32)
    for b in range(B):
        acc = np.zeros((256, L), np.float32)
        for dq in range(4):
            acc += res.results[4 * b + dq]["attnT"]
        out[b] = x[b] + acc.T
    return out.astype(np.float32)


# revision 23
# speedup vs baseline: 1.0217x; 1.0119x over previous
"""Bidirectional Mamba block (nn_Block_bi_mamba) Trainium2 Bass kernel.

Sharding: 8 cores = (batch b in {0,1}) x (d_inner quarter dq in {0..3}).
Each core computes, for its batch and both scan directions, the full
in_proj+conv (folded into PE matmuls) and x_proj (contracts over all 512
channels), the selective scan for its own 128 channels, and the out_proj
partial product [256, L]. The host sums the 4 partials per batch and
adds the residual x. The d_inner axis is permuted per core so the core's
own channel block is always channel-tile 0, making the device program
identical across cores (SPMD) with only input data differing.

Scan cluster runs in a state-major layout: SBUF partition p = 16 states
x 8 channels (channel group g covers channels 8g..8g+7). Per group the
DVE does exactly three ops (dBu mult, tensor_tensor_scan, C mult); the
16-state reduction y = sum_n C_n*h_n + D*u happens on the PE as PSUM-
accumulated selector/diagonal matmuls, keeping the add chain off the
bottleneck DVE. dt/du are computed channel-major once, round-tripped
through DRAM, and replicated across partitions by broadcast DMA. The
causal depthwise conv + in_proj is folded into PE matmuls with silu
fused into the PSUM-drain activation. All scan-cluster data is fp16
(near-1.0 resolution for dA that bf16 lacks; 2-byte operands run DVE
tensor ops at 2x).

Self-contained: hardcodes all shapes; no sibling imports.
"""
import numpy as np
import ml_dtypes
from contextlib import ExitStack

import concourse.bacc as bacc
import concourse.bass as bass
import concourse.tile as tile
from concourse import mybir
from concourse.bass_utils import run_bass_kernel_spmd
from concourse.alu_op_type import AluOpType as CCE

bf = ml_dtypes.bfloat16
FP32 = mybir.dt.float32
BF16 = mybir.dt.bfloat16
FP16 = mybir.dt.float16

B, L = 2, 4096
LC = 2048
NCH = L // LC
NSUB = LC // 512
N = 16
AOP = mybir.AluOpType
AF = mybir.ActivationFunctionType

# wconst fp32 [128, 44] columns
COL = {"A_f": 0, "A_b": 16, "cb_f": 32, "cb_b": 36, "Dd_f": 40,
       "Dd_b": 41, "dtb_f": 42, "dtb_b": 43}


def _bcast_ap(tensor_ap, row0, row_stride_rows, pdims, lc):
    """DRAM source AP replicating rows across 128 partitions.

    pdims: list of [stride_rows, size] partition dims (product 128),
    strides given in rows of length `lc`."""
    base = tensor_ap[row0:row0 + 1, 0:lc]
    ap = [[s * lc, n] for (s, n) in pdims] + [[1, lc]]
    return bass.AP(tensor=base.tensor, offset=base.offset, ap=ap)


def build_program(tc, ins, outs):
    nc = tc.nc
    with ExitStack() as ctx:
        wp = ctx.enter_context(tc.tile_pool(name="wp", bufs=1))
        big = ctx.enter_context(tc.tile_pool(name="big", bufs=1))
        work = ctx.enter_context(tc.tile_pool(name="work", bufs=1))
        scanp = ctx.enter_context(tc.tile_pool(name="scanp", bufs=2))
        ps = ctx.enter_context(tc.tile_pool(name="ps", bufs=2, space="PSUM"))
        dramp = ctx.enter_context(tc.tile_pool(name="dramp", bufs=2,
                                               space="DRAM"))

        # ---- weights ----
        wconst = wp.tile([128, 44], FP32, tag="wconst")
        nc.sync.dma_start(out=wconst, in_=ins["wconst"])

        wconv = wp.tile([128, 16 * 512], BF16, tag="wconv")
        nc.sync.dma_start(out=wconv, in_=ins["wconvP"])
        wz = wp.tile([128, 256], BF16, tag="wz")
        nc.sync.dma_start(out=wz, in_=ins["wzP"])
        outw = wp.tile([128, 256], FP16, tag="outw")
        nc.sync.dma_start(out=outw, in_=ins["outWT"])
        xpro = wp.tile([128, 384], FP16, tag="xpro")
        nc.sync.dma_start(out=xpro, in_=ins["xprojP"])
        dtprojp = wp.tile([16, 256], FP16, tag="dtprojp")
        nc.sync.dma_start(out=dtprojp, in_=ins["dtprojp"])
        selR = wp.tile([128, 16 * 128], FP16, tag="selR")
        nc.sync.dma_start(out=selR, in_=ins["selR"])
        selD = wp.tile([128, 2 * 128], FP16, tag="selD")
        nc.sync.dma_start(out=selD, in_=ins["selD"])
        carry = wp.tile([128, 32], FP32, tag="carry")

        # ---- persistent buffers ----
        xtp = [big.tile([128, L + 6], BF16, tag=f"xtp{kt}", name=f"xtp{kt}")
               for kt in range(2)]
        for kt in range(2):
            nc.sync.dma_start(out=xtp[kt],
                              in_=ins["xT16p"][kt * 128:(kt + 1) * 128])
        zs_all = big.tile([128, L], FP16, tag="zs")
        y_ball = big.tile([128, L], FP16, tag="yball")  # b-dir y, k-space

        # ---- phase Z (emitted as a function; interleaved below) ----
        def phase_z(c):
            for nsub in range(NSUB):
                pt = ps.tile([128, 512], FP32, tag="ps_conv")
                for kt in range(2):
                    nc.tensor.matmul(
                        pt, wz[:, kt * 128:(kt + 1) * 128],
                        xtp[kt][:, 3 + c * LC + nsub * 512:
                                3 + c * LC + (nsub + 1) * 512],
                        start=(kt == 0), stop=(kt == 1))
                nc.scalar.activation(
                    out=zs_all[:, c * LC + nsub * 512:
                               c * LC + (nsub + 1) * 512],
                    in_=pt, func=AF.Silu, bias=0.0, scale=1.0)

        combos = ([("b", c) for c in range(NCH - 1, -1, -1)]
                  + [("f", c) for c in range(NCH)])

        def front_end(d, c):
            """conv+silu -> xc; x_proj -> dbl + DRAM; dt/du -> DRAM."""
            dcol = 0 if d == "f" else 1
            base = 0 if d == "f" else 3
            cb0 = COL[f"cb_{d}"]
            xc = [work.tile([128, LC], FP16, tag=f"xc{t}", name=f"xc{t}",
                            bufs=(2 if t == 0 else 1)) for t in range(4)]
            dbl = work.tile([48, LC], FP16, tag="dbl", bufs=1)
            dt = work.tile([128, LC], FP16, tag="dt", bufs=1)
            # nsub-major so dt is ready as early as possible (the next
            # combo's du-mult sits in the DVE stream waiting for it)
            for nsub in range(NSUB):
                sl = slice(nsub * 512, (nsub + 1) * 512)
                for mt in range(4):
                    pt = ps.tile([128, 512], FP32, tag="ps_conv")
                    ns0 = c * LC + nsub * 512 + base
                    for idx, (k, kt) in enumerate(
                            (k, kt) for k in range(4) for kt in range(2)):
                        seg = ((dcol * 4 + k) * 2 + kt) * 512
                        nc.tensor.matmul(
                            pt, wconv[:, seg + mt * 128:seg + (mt + 1) * 128],
                            xtp[kt][:, ns0 + k:ns0 + k + 512],
                            start=(idx == 0), stop=(idx == 7))
                    # xc = silu(psum + conv_b) in one act
                    nc.scalar.activation(
                        out=xc[mt][:, sl], in_=pt, func=AF.Silu,
                        bias=wconst[:, cb0 + mt:cb0 + mt + 1], scale=1.0)
                pj = ps.tile([48, 512], FP32, tag="ps_xp", bufs=1)
                for kt in range(4):
                    nc.tensor.matmul(
                        pj, xpro[:, kt * 96 + 48 * dcol:
                                 kt * 96 + 48 * (dcol + 1)],
                        xc[kt][:, sl],
                        start=(kt == 0), stop=(kt == 3))
                nc.scalar.copy(out=dbl[:, sl], in_=pj)
                pt = ps.tile([128, 512], FP32, tag="ps_dt", bufs=1)
                nc.tensor.matmul(
                    pt, dtprojp[:, dcol * 128:(dcol + 1) * 128],
                    dbl[0:16, sl],
                    start=True, stop=True)
                esub = work.tile([128, 512], FP16, tag="esub", bufs=1)
                nc.scalar.activation(
                    out=esub, in_=pt, func=AF.Exp,
                    bias=wconst[:, COL[f"dtb_{d}"]:COL[f"dtb_{d}"] + 1],
                    scale=1.0)
                nc.scalar.activation(
                    out=dt[:, sl], in_=esub,
                    func=AF.Ln, bias=1.0, scale=1.0)
            scratch = dramp.tile([32, LC], FP16, tag="bcdram")
            nc.sync.dma_start(out=scratch, in_=dbl[16:48, :])

            dtdram = dramp.tile([128, LC], FP16, tag="dtdram")
            nc.sync.dma_start(out=dtdram, in_=dt)
            st = {"xc0": xc[0], "dt": dt, "scratch": scratch,
                  "dtdram": dtdram}

            def mid_emit():
                """du mult + DRAM write, deferred so the DVE stream of the
                previous combo's scan cluster is not blocked on dt."""
                du = work.tile([128, LC], FP16, tag="du", bufs=1)
                nc.vector.tensor_tensor(du, st["dt"], st["xc0"], AOP.mult)
                dudram = dramp.tile([128, LC], FP16, tag="dudram")
                nc.sync.dma_start(out=dudram, in_=du)
                st["dudram"] = dudram

            st["mid_emit"] = mid_emit
            return st

        def make_bcasts(st):
            """Broadcast cursor: header (B/C tiles) + per-group dt/du
            replication DMAs, emitted incrementally so they can be issued
            ahead across combo boundaries."""
            bc = {"hdr": False, "n": 0, "dtr": {}, "dur": {}}

            def emit(upto):
                if not bc["hdr"]:
                    # B/C rows replicated 8x: partition p=(n,j) <- row n
                    bc["Bt"] = scanp.tile([128, LC], FP16, tag="Bt", bufs=2, name="Bt")
                    nc.sync.dma_start(out=bc["Bt"], in_=_bcast_ap(
                        st["scratch"], 0, 1, [[1, 16], [0, 8]], LC))
                    bc["Ct"] = scanp.tile([128, LC], FP16, tag="Ct", bufs=2, name="Ct")
                    nc.sync.dma_start(out=bc["Ct"], in_=_bcast_ap(
                        st["scratch"], 16, 1, [[1, 16], [0, 8]], LC))
                    bc["hdr"] = True
                while bc["n"] < min(upto, 16):
                    g = bc["n"]
                    # dt/du rows 8g..8g+7 replicated 16x: p=(n,j) <- row 8g+j
                    dtr = scanp.tile([128, LC], FP16, tag="dtr", bufs=4, name="dtr")
                    nc.sync.dma_start(out=dtr, in_=_bcast_ap(
                        st["dtdram"], 8 * g, 1, [[0, 16], [1, 8]], LC))
                    dur = scanp.tile([128, LC], FP16, tag="dur", bufs=4, name="dur")
                    nc.sync.dma_start(out=dur, in_=_bcast_ap(
                        st["dudram"], 8 * g, 1, [[0, 16], [1, 8]], LC))
                    bc["dtr"][g], bc["dur"][g] = dtr, dur
                    bc["n"] += 1

            st["emit_bcasts"] = emit
            st["bc"] = bc

        def scan_phase(d, c, st, first, next_mid=None, post_emit=None,
                       next_st=None):
            dcol = 0 if d == "f" else 1
            rev = (lambda ap: ap[:, ::-1]) if d == "b" else (lambda ap: ap)
            cr = slice(c * LC, (c + 1) * LC)
            st["emit_bcasts"](3)
            bc = st["bc"]
            Bt, Ct = bc["Bt"], bc["Ct"]

            ypsum = ps.tile([128, LC], FP32, tag="ps_y", bufs=1)
            # D*u skip opens the accumulation group (start); b-dir psum is
            # in k-space (time-reversed), so feed u reversed.
            if d == "b":
                u_d = work.tile([128, LC], FP16, tag="xcr", bufs=1)
                nc.scalar.copy(out=u_d, in_=st["xc0"][:, ::-1])
            else:
                u_d = st["xc0"]
            for ns in range(NSUB):
                nc.tensor.matmul(
                    ypsum[:, ns * 512:(ns + 1) * 512],
                    selD[:, dcol * 128:(dcol + 1) * 128],
                    u_d[:, ns * 512:(ns + 1) * 512],
                    start=True, stop=False, skip_group_check=True)
            for g in range(16):
                if g == 9 and next_mid is not None:
                    next_mid()
                if g == 4 and post_emit is not None:
                    post_emit()
                if g == 14 and next_st is not None:
                    next_st["emit_bcasts"](1)
                st["emit_bcasts"](g + 4)
                dtr = bc["dtr"].pop(g)
                dur = bc["dur"].pop(g)

                dA = scanp.tile([128, LC], FP16, tag="dA", bufs=4)
                with tc.high_priority():
                    nc.scalar.activation(
                        out=dA, in_=dtr, func=AF.Exp, bias=0.0,
                        scale=wconst[:, COL[f"A_{d}"] + g:
                                     COL[f"A_{d}"] + g + 1])
                dBu = scanp.tile([128, LC], FP16, tag="dBu", bufs=3)
                nc.vector.tensor_tensor(dBu, dur, Bt, AOP.mult)
                h = scanp.tile([128, LC], FP16, tag="h", bufs=2)
                init = (0.0 if first
                        else carry[:, dcol * 16 + g:dcol * 16 + g + 1])
                nc.vector.tensor_tensor_scan(h, rev(dA), rev(dBu), init,
                                             AOP.mult, AOP.add)
                if first and NCH > 1:
                    nc.vector.tensor_copy(
                        out=carry[:, dcol * 16 + g:dcol * 16 + g + 1],
                        in_=h[:, LC - 1:LC])
                hC = scanp.tile([128, LC], FP16, tag="hC", bufs=3)
                nc.vector.tensor_tensor(hC, h, rev(Ct), AOP.mult)
                # PE: accumulate this group's 16 states into y channels
                for ns in range(NSUB):
                    nc.tensor.matmul(
                        ypsum[:, ns * 512:(ns + 1) * 512],
                        selR[:, g * 128:(g + 1) * 128],
                        hC[:, ns * 512:(ns + 1) * 512],
                        start=False, stop=(g == 15), skip_group_check=True)

            if d == "b":
                # keep k-space; f-tail reads it reversed
                with tc.high_priority():
                    for ns in range(NSUB):
                        nc.scalar.copy(
                            out=y_ball[:, c * LC + ns * 512:
                                       c * LC + (ns + 1) * 512],
                            in_=ypsum[:, ns * 512:(ns + 1) * 512])
            else:
                ysb = work.tile([128, LC], FP16, tag="ysb", bufs=2)
                with tc.high_priority():
                    for ns in range(NSUB):
                        nc.scalar.copy(out=ysb[:, ns * 512:(ns + 1) * 512],
                                       in_=ypsum[:, ns * 512:(ns + 1) * 512])
                nc.vector.tensor_tensor(ysb, ysb, y_ball[:, cr][:, ::-1],
                                        AOP.add)
                nc.vector.tensor_tensor(ysb, ysb, zs_all[:, cr], AOP.mult)
                ygated = ysb
                # out_proj: psum regions cycle inside ypsum
                for mt in range(2):
                    osb = work.tile([128, LC], FP32, tag="osb")
                    for nsub in range(NSUB):
                        q = (mt * NSUB + nsub) % NSUB
                        po = ypsum[:, q * 512:(q + 1) * 512]
                        nc.tensor.matmul(
                            po, outw[:, mt * 128:(mt + 1) * 128],
                            ygated[:, nsub * 512:(nsub + 1) * 512],
                            start=True, stop=True)
                        nc.scalar.copy(
                            out=osb[:, nsub * 512:(nsub + 1) * 512], in_=po)
                    nc.sync.dma_start(
                        out=outs["attnT"][mt * 128:(mt + 1) * 128, cr],
                        in_=osb)

        # software pipeline: front_end one combo ahead of the scan phase;
        # du-mult of combo j+1 and phase-Z are emitted inside scan j's
        # group loop so they never head-block the DVE stream
        states = {}
        states[0] = front_end(*combos[0])
        states[0]["mid_emit"]()
        make_bcasts(states[0])
        for j, (d, c) in enumerate(combos):
            if j + 1 < len(combos):
                states[j + 1] = front_end(*combos[j + 1])
                make_bcasts(states[j + 1])
                next_mid = states[j + 1]["mid_emit"]
                next_st = states[j + 1]
            else:
                next_mid = None
                next_st = None
            post = (lambda cc=j: phase_z(cc)) if j < NCH else None
            first = (j % NCH == 0)
            scan_phase(d, c, states.pop(j), first, next_mid=next_mid,
                       post_emit=post, next_st=next_st)


def build_nc():
    nc = bacc.Bacc("TRN2", target_bir_lowering=False, debug=False,
                   enable_asserts=False)
    ins = {}

    def inp(name, shape, dt):
        ins[name] = nc.dram_tensor(name, shape, dt,
                                   kind="ExternalInput").ap()

    inp("xT16p", [256, L + 6], BF16)
    inp("wconvP", [128, 16 * 512], BF16)
    inp("wzP", [128, 256], BF16)
    inp("outWT", [128, 256], FP16)
    inp("xprojP", [128, 384], FP16)
    inp("dtprojp", [16, 256], FP16)
    inp("wconst", [128, 44], FP32)
    inp("selR", [128, 16 * 128], FP16)
    inp("selD", [128, 2 * 128], FP16)
    outs = {"attnT": nc.dram_tensor("attnT", [256, L], FP32,
                                    kind="ExternalOutput").ap()}
    with tile.TileContext(nc) as tc:
        build_program(tc, ins, outs)
    nc.compile()
    return nc


def prep_core_inputs(inputs, b, dq):
    """Per-core input arrays; d_inner axis permuted so own block is first."""
    own = np.arange(dq * 128, (dq + 1) * 128)
    rest = np.array([i for i in range(512)
                     if not (dq * 128 <= i < (dq + 1) * 128)])
    perm = np.concatenate([own, rest])

    out = {}
    xT = inputs["x"][b].T.astype(np.float32)  # [256, L]
    xTp = np.zeros((256, L + 6), np.float32)
    xTp[:, 3:L + 3] = xT
    out["xT16p"] = xTp.astype(bf)

    w_inx = inputs["in_proj_w"][:512][perm].astype(np.float32)  # [512, 256]
    wconvP = np.zeros((128, 16 * 512), np.float32)
    for dcol, d in enumerate("fb"):
        cw = inputs[f"conv_w_{d}"][:, 0, :][perm].astype(np.float32)
        for k in range(4):
            tap = cw[:, k] if d == "f" else cw[:, 3 - k]
            WdkT = (tap[:, None] * w_inx).T     # [256, 512]
            for kt in range(2):
                seg = ((dcol * 4 + k) * 2 + kt) * 512
                wconvP[:, seg:seg + 512] = WdkT[kt * 128:(kt + 1) * 128]
    out["wconvP"] = wconvP.astype(bf)

    wz = inputs["in_proj_w"][512:1024][own].astype(np.float32)  # [128, 256]
    wzP = np.zeros((128, 256), np.float32)
    for kt in range(2):
        wzP[:, kt * 128:(kt + 1) * 128] = wz.T[kt * 128:(kt + 1) * 128]
    out["wzP"] = wzP.astype(bf)

    out["outWT"] = np.ascontiguousarray(
        inputs["out_proj_w"][:, own].T).astype(np.float16)  # [128, 256]

    xprojP = np.zeros((128, 384), np.float32)
    xpf = inputs["xproj_w_f"][:, perm].T  # [512, 48]
    xpb = inputs["xproj_w_b"][:, perm].T
    for kt in range(4):
        xprojP[:, kt * 96:kt * 96 + 48] = xpf[kt * 128:(kt + 1) * 128]
        xprojP[:, kt * 96 + 48:kt * 96 + 96] = xpb[kt * 128:(kt + 1) * 128]
    out["xprojP"] = xprojP.astype(np.float16)

    out["dtprojp"] = np.ascontiguousarray(np.concatenate(
        [inputs["dtproj_w_f"][own].T, inputs["dtproj_w_b"][own].T],
        axis=1)).astype(np.float16)  # [16, 256]

    # state-major helpers: partition p = n*8 + j  (n = state, j = chan%8)
    pn = np.arange(128) // 8
    pj = np.arange(128) % 8

    wconst = np.zeros((128, 44), np.float32)
    for i, d in enumerate("fb"):
        A = -np.exp(inputs[f"A_log_{d}"][own].astype(np.float64))  # [128, 16]
        for g in range(16):
            wconst[:, 16 * i + g] = A[8 * g + pj, pn]
        cb = inputs[f"conv_b_{d}"][perm]
        wconst[:, 32 + 4 * i:36 + 4 * i] = cb.reshape(4, 128).T
        wconst[:, 40 + i] = inputs[f"D_{d}"][own]
        wconst[:, 42 + i] = inputs[f"dtproj_b_{d}"][own]
    out["wconst"] = wconst

    selR = np.zeros((128, 16 * 128), np.float16)
    for g in range(16):
        selR[np.arange(128), g * 128 + 8 * g + pj] = 1.0
    out["selR"] = selR
    selD = np.zeros((128, 2 * 128), np.float16)
    for i, d in enumerate("fb"):
        selD[np.arange(128), i * 128 + np.arange(128)] = \
            inputs[f"D_{d}"][own].astype(np.float16)
    out["selD"] = selD
    return out


_CACHE = {}


def kernel(**inputs):
    inputs = {k: np.asarray(v) for k, v in inputs.items()}
    if "nc" not in _CACHE:
        _CACHE["nc"] = build_nc()
    nc = _CACHE["nc"]

    core_ids = list(range(8))
    in_maps = [prep_core_inputs(inputs, core // 4, core % 4)
               for core in core_ids]
    import os
    trace = os.environ.get("BASS_KERNEL_TRACE", "0") == "1"
    res = run_bass_kernel_spmd(nc, in_maps, core_ids, trace=trace)
    _CACHE["last_results"] = res

    x = inputs["x"].astype(np.float32)
    out = np.empty((B, L, 256), np.float32)
    for b in range(B):
        acc = np.zeros((256, L), np.float32)
        for dq in range(4):
            acc += res.results[4 * b + dq]["attnT"]
        out[b] = x[b] + acc.T
    return out.astype(np.float32)
